# revision 12
# baseline (speedup 1.0000x reference)
"""Trainium2 Bass kernel for the ConexaoRegional locally-connected layer.

Computation:  z[b, n, d, s] = sum_{h,w} region_n(x[b])[h, w] * pesos[n, d, s, h, w]
  x:     [32, 1, 256, 256] f32
  pesos: [4096, 16, 16, 4, 4] f32
  out:   [32, 4096, 16, 16] f32

Sharding: tensor-parallel over regions (N) across 8 cores. Core c handles
regions [512c, 512c+512) (region-rows [8c, 8c+8), x pixel rows [32c, 32c+32)).

Local region id mapping: n_local = 16*g + 4*i + hg with group g in [0,32),
row-group i in [0,4), col-group hg in [0,4). Groups are processed in pairs
gp = g//2, u = g%2.

All DRAM I/O is bf16 (the correctness gate is rel_err < 2e-2; bf16 in/out
lands ~2e-3). Per-core traffic: x 1MB + W 4MB + out 8MB = 13MB, vs the
~420 GB/s aggregate two-HWDGE-ring DMA ceiling -> ~31us roofline.

Key trick: all matmuls use K=32 with the x operand zero-padded in the half
not belonging to its group parity u (u=0 real in rows 32i..32i+15, u=1 real
in rows 32i+16..32i+31). The zero x rows nullify the other parity's W rows,
so W packs BOTH parities densely across all 128 partitions:
  wsb[32*i + 16*u + k, 1024*gp + 256*hg + ds] = W[n(2gp+u, i, hg), k, ds]
Dense 128-partition DMAs engage all 16 DMA engines.

Per-core dataflow:
  - Loads are chunked and front-loaded on the two HWDGE rings (sync,
    scalar), interleaved so pair gp's x chunk and W chunk land early;
    stores (1MB, two pairs each) are appended after the loads on the same
    rings. Each ring moves 6.5MB; rings drain back-to-back, so reps
    pipeline with no bubble.
  - Per pair gp: 32 matmuls (i, hg, u) on PE tile (32i, 32hg), K=32, bf16:
      psum[32*hg + b, 512*i + 256*u + ds] = sum_k xr^T[k, b] * W[k, ds]
    Row group i owns PSUM bank i; psum pool bufs=2 covers all 8 banks.
  - PSUM -> SBUF evacuation downcasts to bf16: VectorE copies cols 0:1280,
    ScalarE 1280:2048 (scalar is lighter because it also triggers its
    ring's DMAs). Staging is a single [128, 16*2048] bf16 buffer so evac
    never blocks on store completion.
"""

import numpy as np

N_CORES = 8
B = 32
N_REG = 4096
DS = 256
K = 16
RPC = N_REG // N_CORES  # 512
NG = 32                 # groups per core
NP = NG // 2            # group pairs
VCOLS = 1280            # vector's share of the 2048-wide PSUM evacuation

_CACHE = {}


def _build_nc(reps=1, dyn_reps=1):
    import contextlib

    import concourse.bacc as bacc
    import concourse.mybir as mybir
    import concourse.tile as tile

    F32 = mybir.dt.float32
    BF16 = mybir.dt.bfloat16
    nc = bacc.Bacc("TRN2", target_bir_lowering=False, debug=False)
    x_d = nc.dram_tensor("x", [128, NP, 256], BF16, kind="ExternalInput")
    w_d = nc.dram_tensor("w", [128, NP, 1024], BF16, kind="ExternalInput")
    o_d = nc.dram_tensor("o", [128, NP, 2048], BF16, kind="ExternalOutput")

    with tile.TileContext(nc) as tc:
        with (
            tc.tile_pool(name="xsb", bufs=1) as xp,
            tc.tile_pool(name="wsb", bufs=1) as wp,
            tc.tile_pool(name="ostage", bufs=1) as op,
            tc.tile_pool(name="pso", bufs=2, space="PSUM") as psop,
        ):
            xsb = xp.tile([128, NG * 128], BF16)
            wsb = wp.tile([128, NP * 1024], BF16)
            osb = op.tile([128, NP * 2048], BF16)
            xflat = x_d.ap().rearrange("p gp f -> p (gp f)")
            wflat = w_d.ap().rearrange("p gp f -> p (gp f)")
            oflat = o_d.ap().rearrange("p gp f -> p (gp f)")

            loop_cm = (
                tc.For_i(0, dyn_reps, 1)
                if dyn_reps > 1
                else contextlib.nullcontext()
            )
            with loop_cm:
                for rep in range(reps):
                    _one_rep(nc, xflat, wflat, oflat, xsb, wsb, osb, psop, F32)

    nc.compile()
    return nc


def _one_rep(nc, xflat, wflat, oflat, xsb, wsb, osb, psop, F32):
    # Front-loaded loads, bytes balanced 2.5MB/ring. ("x", q) loads x pairs
    # [4q, 4q+4) = xsb cols [1024q, 1024q+1024); ("w", gp) loads W pairs
    # {gp, gp+1} = wsb cols [1024gp, 1024(gp+2)).
    ring_a = [("x", 0), ("w", 0), ("w", 4), ("x", 2), ("w", 8), ("w", 12)]
    ring_b = [("w", 2), ("x", 1), ("w", 6), ("x", 3), ("w", 10), ("w", 14)]
    for eng, ring in ((nc.sync, ring_a), (nc.scalar, ring_b)):
        for kind, q in ring:
            if kind == "x":
                eng.dma_start(
                    out=xsb[:, 1024 * q : 1024 * (q + 1)],
                    in_=xflat[:, 1024 * q : 1024 * (q + 1)],
                )
            else:
                eng.dma_start(
                    out=wsb[:, 1024 * q : 1024 * (q + 2)],
                    in_=wflat[:, 1024 * q : 1024 * (q + 2)],
                )
    for gp in range(NP):
        # Two PSUM half-tiles per pair (i in {0,1} -> lo, {2,3} -> hi), so
        # all 8 banks hold two pairs in flight and the two evac engines work
        # concurrently on every pair (vector lo, scalar hi).
        psum_lo = psop.tile([128, 1024], F32)
        psum_hi = psop.tile([128, 1024], F32)
        for i in range(4):
            half = psum_lo if i < 2 else psum_hi
            base = 512 * (i % 2)
            for hg in range(4):
                for u in range(2):
                    g = 2 * gp + u
                    nc.tensor.matmul(
                        half[
                            32 * hg : 32 * hg + 32,
                            base + 256 * u : base + 256 * u + 256,
                        ],
                        xsb[
                            32 * i : 32 * i + 32,
                            128 * g + 32 * hg : 128 * g + 32 * hg + 32,
                        ],
                        wsb[
                            32 * i : 32 * i + 32,
                            1024 * gp + 256 * hg : 1024 * gp + 256 * hg + 256,
                        ],
                        start=True,
                        stop=True,
                        tile_position=(32 * i, 32 * hg),
                    )
        ost = osb[:, 2048 * gp : 2048 * (gp + 1)]
        nc.vector.tensor_copy(out=ost[:, :1024], in_=psum_lo[:])
        nc.scalar.copy(out=ost[:, 1024:], in_=psum_hi[:])
        if gp % 2 == 1:
            # 1MB two-pair store; blocks alternate rings
            store_eng = nc.sync if gp % 4 == 1 else nc.scalar
            store_eng.dma_start(
                out=oflat[:, 2048 * (gp - 1) : 2048 * (gp + 1)],
                in_=osb[:, 2048 * (gp - 1) : 2048 * (gp + 1)],
            )


def _prep_in_maps(x, pesos):
    """Full inputs -> list of 8 per-core input dicts (host-side layout prep)."""
    import ml_dtypes

    bf16 = np.dtype(ml_dtypes.bfloat16)
    x = np.asarray(x, dtype=np.float32)
    pesos = np.asarray(pesos, dtype=np.float32)
    # pesos [n, d, s, h, w] -> [n, k=(h*4+w), ds=(d*16+s)]
    pesos_t = np.ascontiguousarray(pesos.transpose(0, 3, 4, 1, 2)).reshape(
        N_REG, K, DS
    )
    in_maps = []
    for c in range(N_CORES):
        # x regions for this core: [b, n_local, k]
        x_c = x[:, 0, 32 * c : 32 * c + 32, :]
        xr = (
            x_c.reshape(B, 8, 4, 64, 4)
            .transpose(0, 1, 3, 2, 4)
            .reshape(B, RPC, K)
        )
        # xt[32i + 16u + k, g, 32hg + b] = xr[b, 16g+4i+hg, k] if g%2==u else 0
        a = xr.reshape(B, NG, 4, 4, K).transpose(2, 4, 1, 3, 0)  # i,k,g,hg,b
        xt = np.zeros((4, 2, K, NG, 4, B), dtype=bf16)
        xt[:, 0, :, 0::2] = a[:, :, 0::2].astype(bf16)
        xt[:, 1, :, 1::2] = a[:, :, 1::2].astype(bf16)
        xt = xt.reshape(128, NP, 256)

        # w[32i + 16u + k, gp, 256hg + ds] = pesos_t[512c + 16(2gp+u)+4i+hg, k, ds]
        wc = pesos_t[512 * c : 512 * (c + 1)].reshape(NP, 2, 4, 4, K, DS)
        w_arr = np.ascontiguousarray(
            wc.transpose(2, 1, 4, 0, 3, 5).astype(bf16)  # i, u, k, gp, hg, ds
        ).reshape(128, NP, 1024)

        in_maps.append({"x": np.ascontiguousarray(xt), "w": w_arr})
    return in_maps


def _unshard(results):
    """Per-core outputs -> full [B, N, 16, 16]."""
    out = np.empty((B, N_REG, DS), dtype=np.float32)
    for c, res in enumerate(results):
        o_c = np.asarray(res["o"]).astype(np.float32)
        # o[32hg + b, gp, 512i + 256u + ds]
        o_c = o_c.reshape(4, B, NP, 4, 2, DS)   # hg, b, gp, i, u, ds
        o_t = o_c.transpose(1, 2, 4, 3, 0, 5)   # b, gp, u, i, hg, ds
        out[:, 512 * c : 512 * (c + 1), :] = o_t.reshape(B, RPC, DS)
    return out.reshape(B, N_REG, 16, 16)


def kernel(x, pesos):
    from concourse.bass_utils import run_bass_kernel_spmd

    if "nc" not in _CACHE:
        _CACHE["nc"] = _build_nc()
    nc = _CACHE["nc"]
    in_maps = _prep_in_maps(x, pesos)
    res = run_bass_kernel_spmd(nc, in_maps, core_ids=list(range(N_CORES)))
    return _unshard(res.results)


# revision 16
# speedup vs baseline: 1.0073x; 1.0073x over previous
"""Trainium2 Bass kernel for the ConexaoRegional locally-connected layer.

Computation:  z[b, n, d, s] = sum_{h,w} region_n(x[b])[h, w] * pesos[n, d, s, h, w]
  x:     [32, 1, 256, 256] f32
  pesos: [4096, 16, 16, 4, 4] f32
  out:   [32, 4096, 16, 16] f32

Sharding: tensor-parallel over regions (N) across 8 cores. Core c handles
regions [512c, 512c+512) (region-rows [8c, 8c+8), x pixel rows [32c, 32c+32)).

Local region id mapping: n_local = 16*g + 4*i + hg with group g in [0,32),
row-group i in [0,4), col-group hg in [0,4). Groups are processed in pairs
gp = g//2, u = g%2.

All DRAM I/O is bf16 (the correctness gate is rel_err < 2e-2; bf16 in/out
lands ~2e-3). Per-core traffic: x 1MB + W 4MB + out 8MB = 13MB, vs the
~420 GB/s aggregate two-HWDGE-ring DMA ceiling -> ~31us roofline.

Key trick: all matmuls use K=32 with the x operand zero-padded in the half
not belonging to its group parity u (u=0 real in rows 32i..32i+15, u=1 real
in rows 32i+16..32i+31). The zero x rows nullify the other parity's W rows,
so W packs BOTH parities densely across all 128 partitions:
  wsb[32*i + 16*u + k, 1024*gp + 256*hg + ds] = W[n(2gp+u, i, hg), k, ds]
Dense 128-partition DMAs engage all 16 DMA engines.

Per-core dataflow:
  - Loads are chunked and front-loaded on the two HWDGE rings (sync,
    scalar), interleaved so pair gp's x chunk and W chunk land early;
    stores (1MB, two pairs each) are appended after the loads on the same
    rings. Each ring moves 6.5MB; rings drain back-to-back, so reps
    pipeline with no bubble.
  - Per pair gp: 32 matmuls (i, hg, u) on PE tile (32i, 32hg), K=32, bf16:
      psum[32*hg + b, 512*i + 256*u + ds] = sum_k xr^T[k, b] * W[k, ds]
    Row group i owns PSUM bank i; psum pool bufs=2 covers all 8 banks.
  - PSUM -> SBUF evacuation downcasts to bf16: VectorE copies cols 0:1280,
    ScalarE 1280:2048 (scalar is lighter because it also triggers its
    ring's DMAs). Staging is a single [128, 16*2048] bf16 buffer so evac
    never blocks on store completion.
"""

import numpy as np

N_CORES = 8
B = 32
N_REG = 4096
DS = 256
K = 16
RPC = N_REG // N_CORES  # 512
NG = 32                 # groups per core
NP = NG // 2            # group pairs
VCOLS = 1280            # vector's share of the 2048-wide PSUM evacuation

_CACHE = {}


def _build_nc(reps=1, dyn_reps=1):
    import contextlib

    import concourse.bacc as bacc
    import concourse.mybir as mybir
    import concourse.tile as tile

    F32 = mybir.dt.float32
    BF16 = mybir.dt.bfloat16
    nc = bacc.Bacc("TRN2", target_bir_lowering=False, debug=False)
    x_d = nc.dram_tensor("x", [128, NP, 256], BF16, kind="ExternalInput")
    w_d = nc.dram_tensor("w", [128, NP, 1024], BF16, kind="ExternalInput")
    o_d = nc.dram_tensor("o", [128, NP, 2048], BF16, kind="ExternalOutput")

    with tile.TileContext(nc) as tc:
        with (
            tc.tile_pool(name="xsb", bufs=3) as xp,
            tc.tile_pool(name="wsb", bufs=3) as wp,
            tc.tile_pool(name="ostage", bufs=1) as op,
            tc.tile_pool(name="pso", bufs=2, space="PSUM") as psop,
        ):
            osb = op.tile([128, NP * 2048], BF16)
            xflat = x_d.ap().rearrange("p gp f -> p (gp f)")
            wflat = w_d.ap().rearrange("p gp f -> p (gp f)")
            oflat = o_d.ap().rearrange("p gp f -> p (gp f)")

            loop_cm = (
                tc.For_i(0, dyn_reps, 1)
                if dyn_reps > 1
                else contextlib.nullcontext()
            )
            with loop_cm:
                for rep in range(reps):
                    # fresh tiles per rep: bufs=2 double-buffers loads
                    # across reps so next-rep prefetch never WAR-waits
                    xsb = xp.tile([128, NG * 128], BF16)
                    wsb = wp.tile([128, NP * 1024], BF16)
                    _one_rep(nc, xflat, wflat, oflat, xsb, wsb, osb, psop, F32, BF16)

    nc.compile()
    return nc


def _one_rep(nc, xflat, wflat, oflat, xsb, wsb, osb, psop, F32, BF16):
    # Dedicated rings: sync carries ALL loads (5MB/rep), scalar ALL stores
    # (8MB/rep). Loads never queue behind stores in a ring FIFO, so with
    # double-buffered x/W the load ring streams a full rep ahead and compute
    # never waits on loads mid-rep. ("x", q) loads x pairs [4q, 4q+4) =
    # xsb cols [1024q, +1024); ("w", gp) loads W pairs {gp, gp+1}.
    loads = [
        ("x", 0), ("w", 0), ("w", 2), ("x", 1), ("w", 4), ("w", 6),
        ("x", 2), ("w", 8), ("w", 10), ("x", 3), ("w", 12), ("w", 14),
    ]
    for kind, q in loads:
        if kind == "x":
            nc.sync.dma_start(
                out=xsb[:, 1024 * q : 1024 * (q + 1)],
                in_=xflat[:, 1024 * q : 1024 * (q + 1)],
            )
        else:
            nc.sync.dma_start(
                out=wsb[:, 1024 * q : 1024 * (q + 2)],
                in_=wflat[:, 1024 * q : 1024 * (q + 2)],
            )
    for gp in range(NP):
        # Two PSUM half-tiles per pair (i in {0,1} -> lo, {2,3} -> hi), so
        # all 8 banks hold two pairs in flight and the two evac engines work
        # concurrently on every pair (vector lo, scalar hi).
        psum_lo = psop.tile([128, 1024], F32)
        psum_hi = psop.tile([128, 1024], F32)
        for i in range(4):
            half = psum_lo if i < 2 else psum_hi
            base = 512 * (i % 2)
            for hg in range(4):
                for u in range(2):
                    g = 2 * gp + u
                    nc.tensor.matmul(
                        half[
                            32 * hg : 32 * hg + 32,
                            base + 256 * u : base + 256 * u + 256,
                        ],
                        xsb[
                            32 * i : 32 * i + 32,
                            128 * g + 32 * hg : 128 * g + 32 * hg + 32,
                        ],
                        wsb[
                            32 * i : 32 * i + 32,
                            1024 * gp + 256 * hg : 1024 * gp + 256 * hg + 256,
                        ],
                        start=True,
                        stop=True,
                        tile_position=(32 * i, 32 * hg),
                    )
        ost = osb[:, 2048 * gp : 2048 * (gp + 1)]
        nc.vector.tensor_copy(out=ost[:, :1024], in_=psum_lo[:])
        nc.scalar.copy(out=ost[:, 1024:], in_=psum_hi[:])
        if gp % 2 == 1:
            # 1MB two-pair store on the dedicated store ring (scalar)
            nc.scalar.dma_start(
                out=oflat[:, 2048 * (gp - 1) : 2048 * (gp + 1)],
                in_=osb[:, 2048 * (gp - 1) : 2048 * (gp + 1)],
            )


def _prep_in_maps(x, pesos):
    """Full inputs -> list of 8 per-core input dicts (host-side layout prep)."""
    import ml_dtypes

    bf16 = np.dtype(ml_dtypes.bfloat16)
    x = np.asarray(x, dtype=np.float32)
    pesos = np.asarray(pesos, dtype=np.float32)
    # pesos [n, d, s, h, w] -> [n, k=(h*4+w), ds=(d*16+s)]
    pesos_t = np.ascontiguousarray(pesos.transpose(0, 3, 4, 1, 2)).reshape(
        N_REG, K, DS
    )
    in_maps = []
    for c in range(N_CORES):
        # x regions for this core: [b, n_local, k]
        x_c = x[:, 0, 32 * c : 32 * c + 32, :]
        xr = (
            x_c.reshape(B, 8, 4, 64, 4)
            .transpose(0, 1, 3, 2, 4)
            .reshape(B, RPC, K)
        )
        # xt[32i + 16u + k, g, 32hg + b] = xr[b, 16g+4i+hg, k] if g%2==u else 0
        a = xr.reshape(B, NG, 4, 4, K).transpose(2, 4, 1, 3, 0)  # i,k,g,hg,b
        xt = np.zeros((4, 2, K, NG, 4, B), dtype=bf16)
        xt[:, 0, :, 0::2] = a[:, :, 0::2].astype(bf16)
        xt[:, 1, :, 1::2] = a[:, :, 1::2].astype(bf16)
        xt = xt.reshape(128, NP, 256)

        # w[32i + 16u + k, gp, 256hg + ds] = pesos_t[512c + 16(2gp+u)+4i+hg, k, ds]
        wc = pesos_t[512 * c : 512 * (c + 1)].reshape(NP, 2, 4, 4, K, DS)
        w_arr = np.ascontiguousarray(
            wc.transpose(2, 1, 4, 0, 3, 5).astype(bf16)  # i, u, k, gp, hg, ds
        ).reshape(128, NP, 1024)

        in_maps.append({"x": np.ascontiguousarray(xt), "w": w_arr})
    return in_maps


def _unshard(results):
    """Per-core outputs -> full [B, N, 16, 16]."""
    out = np.empty((B, N_REG, DS), dtype=np.float32)
    for c, res in enumerate(results):
        o_c = np.asarray(res["o"]).astype(np.float32)
        # o[32hg + b, gp, 512i + 256u + ds]
        o_c = o_c.reshape(4, B, NP, 4, 2, DS)   # hg, b, gp, i, u, ds
        o_t = o_c.transpose(1, 2, 4, 3, 0, 5)   # b, gp, u, i, hg, ds
        out[:, 512 * c : 512 * (c + 1), :] = o_t.reshape(B, RPC, DS)
    return out.reshape(B, N_REG, 16, 16)


def kernel(x, pesos):
    from concourse.bass_utils import run_bass_kernel_spmd

    if "nc" not in _CACHE:
        _CACHE["nc"] = _build_nc()
    nc = _CACHE["nc"]
    in_maps = _prep_in_maps(x, pesos)
    res = run_bass_kernel_spmd(nc, in_maps, core_ids=list(range(N_CORES)))
    return _unshard(res.results)
